# revision 6
# baseline (speedup 1.0000x reference)
"""2-layer GCN on 8 trn2 NeuronCores.

- Nodes sharded 8 ways (12500/core, padded 12544). Edges partitioned by target
  core, self-loops appended as ordinary edges; all GCN norms folded into
  per-node scalings (host prescales x by dinv; the one-hot aggregation operand
  S carries dinv[target]; layer 2 aggregates 16-dim using associativity).
- Per-core targets degree-sorted, packed into groups of 32 slots; per-group
  tile budgets are max over cores so one SPMD program serves all cores. Host
  un-permutes the final output.
- Gather: batched indirect DMA from an all-gathered bf16 node table in DRAM.
- Scatter-add: TensorE matmuls (messages stationary, one-hot S moving)
  accumulating agg^T in PSUM.
"""

import math
import numpy as np
import ml_dtypes

import concourse.bacc as bacc
import concourse.tile as tile
from concourse import mybir
from concourse.bass import IndirectOffsetOnAxis
from concourse.bass_utils import run_bass_kernel_spmd
from concourse.masks import make_identity

BF16 = mybir.dt.bfloat16
F32 = mybir.dt.float32
I32 = mybir.dt.int32

N_NODES = 100000
IN_CH, HID, OUT_CH = 256, 16, 40
NCORES = 8
SHARD = N_NODES // NCORES          # 12500
PAD = 12544                        # 98*128
NT_X = PAD // 128                  # 98
GRP = 32                           # targets per slot-group
NGRP = PAD // GRP                  # 392
GPB = 15                           # groups per PSUM bank (480 cols)
NBANK = math.ceil(NGRP / GPB)      # 27
GB = 128                           # tiles per gather batch

_cache = {}


def _host_prep(x, edge_index, W1, b1, W2, b2):
    row = np.asarray(edge_index[0], dtype=np.int64)
    col = np.asarray(edge_index[1], dtype=np.int64)
    deg = np.bincount(col, minlength=N_NODES).astype(np.float64) + 1.0
    dinv = (1.0 / np.sqrt(deg)).astype(np.float32)
    xs = np.asarray(x, np.float32) * dinv[:, None]

    cores = []
    for c in range(NCORES):
        LO = c * SHARD
        m = (col >= LO) & (col < LO + SHARD)
        r_c = np.concatenate([row[m], np.arange(LO, LO + SHARD, dtype=np.int64)])
        t_c = np.concatenate([col[m] - LO, np.arange(SHARD, dtype=np.int64)])
        dl = np.bincount(t_c, minlength=SHARD)
        order = np.argsort(-dl, kind="stable").astype(np.int64)
        slot_of = np.empty(SHARD, np.int64)
        slot_of[order] = np.arange(SHARD)
        key = slot_of[t_c]
        o = np.argsort(key, kind="stable")
        r_c, t_c, key = r_c[o], t_c[o], key[o]
        gid = key // GRP
        egc = np.bincount(gid, minlength=NGRP)
        cores.append(dict(LO=LO, r=r_c, t=t_c, key=key, gid=gid, egc=egc,
                          order=order, slot_of=slot_of))

    TB = np.maximum(1, np.ceil(
        np.stack([c["egc"] for c in cores]).max(0) / 128.0)).astype(np.int64)
    tstart = np.concatenate([[0], np.cumsum(TB)]).astype(np.int64)
    T = int(tstart[-1])
    TPAD = ((T + GB - 1) // GB) * GB

    banks = []
    for b in range(NBANK):
        glo, ghi = b * GPB, min((b + 1) * GPB, NGRP)
        banks.append((glo, ghi, int(tstart[glo]), int(tstart[ghi]), (ghi - glo) * GRP))

    slotpos = np.stack([c["slot_of"] for c in cores])  # [8, SHARD]
    per_core = []
    for c in cores:
        ne = len(c["r"])
        src = np.zeros(T * 128, np.int64)
        sval = np.zeros(T * 128, np.float32)
        sslot = np.zeros(T * 128, np.int64)
        off = np.concatenate([[0], np.cumsum(c["egc"])])
        pos = tstart[c["gid"]] * 128 + (np.arange(ne) - off[c["gid"]])
        src[pos] = c["r"]
        sval[pos] = dinv[c["t"] + c["LO"]]
        sslot[pos] = c["key"] % GRP
        src_tp = src.reshape(T, 128).T
        sv_tp = sval.reshape(T, 128).T
        ss_tp = sslot.reshape(T, 128).T
        cu = src_tp // SHARD
        ru = src_tp % SHARD
        idx1 = (cu * PAD + ru).astype(np.int32)
        idx2 = (cu * PAD + slotpos[cu, ru]).astype(np.int32)
        S = np.zeros((128, T, GRP), np.float32)
        S[np.arange(128)[:, None], np.arange(T)[None, :], ss_tp] = sv_tp
        S = S.reshape(128, T * GRP).astype(ml_dtypes.bfloat16)
        if TPAD > T:
            z = np.zeros((128, TPAD - T), np.int32)
            idx1 = np.concatenate([idx1, z], 1)
            idx2 = np.concatenate([idx2, z], 1)
        dv = np.zeros(PAD, np.float32)
        dv[:SHARD] = dinv[c["order"] + c["LO"]]
        dslot = np.repeat(dv[None, :], HID, 0).astype(np.float32)
        xtT = np.zeros((IN_CH, PAD), np.float32)
        xtT[:, :SHARD] = xs[c["LO"]:c["LO"] + SHARD].T
        xt = xtT.reshape(IN_CH, NT_X, 128).transpose(1, 0, 2)
        per_core.append(dict(
            xt=np.ascontiguousarray(xt).astype(ml_dtypes.bfloat16),
            sarr=S, idx1=idx1, idx2=idx2, dslot=dslot, order=c["order"]))
    shared = dict(
        w1=np.asarray(W1, np.float32).astype(ml_dtypes.bfloat16),
        w2=np.asarray(W2, np.float32).astype(ml_dtypes.bfloat16),
        b1=np.asarray(b1, np.float32).reshape(HID, 1),
        b2r=np.repeat(np.asarray(b2, np.float32).reshape(1, OUT_CH), 128, 0),
    )
    return per_core, shared, T, TPAD, banks, tstart


def _build(T, TPAD, banks, tstart, phase):
    nc = bacc.Bacc("TRN2", target_bir_lowering=False, debug=False, num_devices=NCORES)
    xt = nc.dram_tensor("xt", [NT_X, IN_CH, 128], BF16, kind="ExternalInput").ap()
    w1 = nc.dram_tensor("w1", [IN_CH, HID], BF16, kind="ExternalInput").ap()
    w2 = nc.dram_tensor("w2", [HID, OUT_CH], BF16, kind="ExternalInput").ap()
    b1 = nc.dram_tensor("b1", [HID, 1], F32, kind="ExternalInput").ap()
    b2r = nc.dram_tensor("b2r", [128, OUT_CH], F32, kind="ExternalInput").ap()
    dslot = nc.dram_tensor("dslot", [HID, PAD], F32, kind="ExternalInput").ap()
    sarr = nc.dram_tensor("sarr", [128, T * GRP], BF16, kind="ExternalInput").ap()
    idx1 = nc.dram_tensor("idx1", [128, TPAD], I32, kind="ExternalInput").ap()
    idx2 = nc.dram_tensor("idx2", [128, TPAD], I32, kind="ExternalInput").ap()
    if phase == "A":
        t1l = nc.dram_tensor("t1l", [PAD, HID], BF16)
        t1f = nc.dram_tensor("t1f", [NCORES * PAD, HID], BF16)
        t2l = nc.dram_tensor("t2l", [PAD, HID], BF16, kind="ExternalOutput")
        out = None
    else:
        out = nc.dram_tensor("out", [PAD, OUT_CH], F32, kind="ExternalOutput").ap()
        t2f = nc.dram_tensor("t2f", [NCORES * PAD, HID], BF16, kind="ExternalInput")

    with tile.TileContext(nc) as tc:
        with tc.tile_pool(name="persist", bufs=1) as pp:
            w1a = pp.tile([128, HID], BF16); nc.sync.dma_start(w1a[:], w1[0:128, :])
            w1b = pp.tile([128, HID], BF16); nc.sync.dma_start(w1b[:], w1[128:256, :])
            w2sb = pp.tile([HID, OUT_CH], BF16); nc.sync.dma_start(w2sb[:], w2)
            b1sb = pp.tile([HID, 1], F32); nc.sync.dma_start(b1sb[:], b1)
            b2sb = pp.tile([128, OUT_CH], F32); nc.sync.dma_start(b2sb[:], b2r)
            dsb = pp.tile([HID, PAD], F32); nc.sync.dma_start(dsb[:], dslot)
            ix1 = pp.tile([128, TPAD], I32); nc.sync.dma_start(ix1[:], idx1)
            ix2 = pp.tile([128, TPAD], I32); nc.sync.dma_start(ix2[:], idx2)
            id16 = pp.tile([HID, HID], BF16); make_identity(nc, id16[:])
            id40 = pp.tile([OUT_CH, OUT_CH], BF16); make_identity(nc, id40[:])
            zer16 = pp.tile([128, HID], BF16); nc.vector.memset(zer16[:], 0.0)
            junk = pp.tile([128, GPB * GRP], BF16); nc.vector.memset(junk[:], 0.0)

            # ---- Phase 1: h~ = x~ @ W1 -> bf16 table t1l ----
            if phase == "B":
                agg_layer_holder = []
            if phase == "A":
              with (
                tc.tile_pool(name="xp", bufs=4) as xp,
                tc.tile_pool(name="hp", bufs=3) as hp,
                tc.tile_pool(name="p1ps", bufs=2, space="PSUM") as p1ps,
              ):
                for t in range(NT_X):
                    xa = xp.tile([128, 128], BF16)
                    nc.sync.dma_start(xa[:], xt[t, 0:128, :])
                    xb = xp.tile([128, 128], BF16)
                    nc.sync.dma_start(xb[:], xt[t, 128:256, :])
                    ps = p1ps.tile([128, HID], F32, space="PSUM")
                    nc.tensor.matmul(ps[:], lhsT=xa[:], rhs=w1a[:], start=True, stop=False)
                    nc.tensor.matmul(ps[:], lhsT=xb[:], rhs=w1b[:], start=False, stop=True)
                    hb = hp.tile([128, HID], BF16)
                    nc.scalar.copy(hb[:], ps[:])
                    nc.sync.dma_start(t1l[t * 128:(t + 1) * 128, :], hb[:])

              nc.gpsimd.collective_compute(
                "AllGather", mybir.AluOpType.bypass,
                replica_groups=[list(range(NCORES))],
                ins=[t1l.ap().opt()], outs=[t1f.ap().opt()])

            def agg_layer(tf, ix, is_l1):
                with (
                    tc.tile_pool(name="gp", bufs=8) as gp,
                    tc.tile_pool(name="sp", bufs=3) as sp,
                    tc.tile_pool(name="agg", bufs=3, space="PSUM") as aggp,
                    tc.tile_pool(name="tp", bufs=2, space="PSUM") as tpp,
                    tc.tile_pool(name="ev", bufs=2) as evp,
                    tc.tile_pool(name="tb", bufs=3) as tbp,
                    tc.tile_pool(name="l2p", bufs=2, space="PSUM") as l2p,
                    tc.tile_pool(name="l2s", bufs=4) as l2s,
                ):
                    gbufs, sbufs = {}, {}

                    def ensure_batch(t):
                        gb = gp.tile([128, HID], BF16)
                        nc.gpsimd.indirect_dma_start(
                            out=gb[:], out_offset=None, in_=tf.ap(),
                            in_offset=IndirectOffsetOnAxis(
                                ap=ix[:, t:t + 1], axis=0))
                        gbufs[t] = gb
                        g = t // GB
                        if g not in sbufs:
                            sb = sp.tile([128, GB * GRP], BF16)
                            hi = min((g + 1) * GB * GRP, T * GRP)
                            w = hi - g * GB * GRP
                            nc.sync.dma_start(sb[:, 0:w], sarr[:, g * GB * GRP:hi])
                            sbufs[g] = sb

                    grp_of = np.searchsorted(tstart, np.arange(T), side="right") - 1

                    for (glo, ghi, tlo, thi, width) in banks:
                        ag = aggp.tile([HID, GPB * GRP], F32, space="PSUM")
                        nc.tensor.matmul(ag[:, 0:width], lhsT=zer16[:],
                                         rhs=junk[:, 0:width], start=True, stop=True)
                        for t in range(tlo, thi):
                            g = t // GB
                            ensure_batch(t)
                            cg = (int(grp_of[t]) - glo) * GRP
                            to = t - g * GB
                            nc.tensor.matmul(
                                ag[:, cg:cg + GRP],
                                lhsT=gbufs.pop(t)[:],
                                rhs=sbufs[g][:, to * GRP:(to + 1) * GRP],
                                start=False, stop=True)
                        base = glo * GRP
                        if is_l1:
                            ev = evp.tile([HID, GPB * GRP], F32)
                            nc.scalar.activation(ev[:, 0:width], ag[:, 0:width],
                                                 mybir.ActivationFunctionType.Relu,
                                                 bias=b1sb[:])
                            zt = evp.tile([HID, GPB * GRP], BF16)
                            nc.vector.tensor_tensor(zt[:, 0:width], ev[:, 0:width],
                                                    dsb[:, base:base + width],
                                                    op=mybir.AluOpType.mult)
                            o = 0
                            while o < width:
                                w = min(120, width - o)
                                tp = tpp.tile([120, HID], BF16, space="PSUM")
                                nc.tensor.matmul(tp[0:w, :], lhsT=zt[:, o:o + w],
                                                 rhs=id16[:], is_transpose=True)
                                tb = tbp.tile([120, HID], BF16)
                                nc.scalar.copy(tb[0:w, :], tp[0:w, :])
                                nc.sync.dma_start(t2l[base + o:base + o + w, :], tb[0:w, :])
                                o += w
                        else:
                            rb = evp.tile([HID, GPB * GRP], BF16)
                            nc.scalar.copy(rb[:, 0:width], ag[:, 0:width])
                            o40 = l2p.tile([OUT_CH, GPB * GRP], F32, space="PSUM")
                            nc.tensor.matmul(o40[:, 0:width], lhsT=w2sb[:],
                                             rhs=rb[:, 0:width], start=True, stop=True)
                            c40 = l2s.tile([OUT_CH, GPB * GRP], BF16)
                            nc.scalar.copy(c40[:, 0:width], o40[:, 0:width])
                            o = 0
                            while o < width:
                                w = min(120, width - o)
                                tp = tpp.tile([120, OUT_CH], BF16, space="PSUM")
                                nc.tensor.matmul(tp[0:w, :], lhsT=c40[:, o:o + w],
                                                 rhs=id40[:], is_transpose=True)
                                y = l2s.tile([120, OUT_CH], F32)
                                nc.vector.tensor_tensor(y[0:w, :], tp[0:w, :], b2sb[0:w, :],
                                                        op=mybir.AluOpType.add)
                                mneg = l2s.tile([120, 1], F32)
                                nc.vector.tensor_reduce(mneg[0:w, :], y[0:w, :],
                                                        axis=mybir.AxisListType.X,
                                                        op=mybir.AluOpType.max)
                                nc.vector.tensor_scalar(mneg[0:w, :], mneg[0:w, :], -1.0,
                                                        None, op0=mybir.AluOpType.mult)
                                e = l2s.tile([120, OUT_CH], F32)
                                nc.scalar.activation(e[0:w, :], y[0:w, :],
                                                     mybir.ActivationFunctionType.Exp,
                                                     bias=mneg[0:w, :])
                                sm = l2s.tile([120, 1], F32)
                                nc.vector.tensor_reduce(sm[0:w, :], e[0:w, :],
                                                        axis=mybir.AxisListType.X,
                                                        op=mybir.AluOpType.add)
                                ls = l2s.tile([120, 1], F32)
                                nc.scalar.activation(ls[0:w, :], sm[0:w, :],
                                                     mybir.ActivationFunctionType.Ln)
                                c1 = l2s.tile([120, 1], F32)
                                nc.vector.tensor_tensor(c1[0:w, :], mneg[0:w, :], ls[0:w, :],
                                                        op=mybir.AluOpType.subtract)
                                of = l2s.tile([120, OUT_CH], F32)
                                nc.vector.tensor_tensor(
                                    of[0:w, :], y[0:w, :],
                                    c1[0:w, 0:1].to_broadcast([w, OUT_CH]),
                                    op=mybir.AluOpType.add)
                                nc.sync.dma_start(out[base + o:base + o + w, :], of[0:w, :])
                                o += w

            if phase == "A":
                agg_layer(t1f, ix1, True)
            else:
                agg_layer(t2f, ix2, False)

    nc.compile()
    return nc


def kernel(x, edge_index, W1, b1, W2, b2):
    per_core, shared, T, TPAD, banks, tstart = _host_prep(x, edge_index, W1, b1, W2, b2)
    key = (T, TPAD, tuple(tstart.tolist()))
    if key not in _cache:
        _cache[key] = (_build(T, TPAD, banks, tstart, "A"),
                       _build(T, TPAD, banks, tstart, "B"))
    ncA, ncB = _cache[key]

    def maps(extra):
        ms = []
        for c in range(NCORES):
            pc = per_core[c]
            m = {"xt": pc["xt"], "w1": shared["w1"], "w2": shared["w2"],
                 "b1": shared["b1"], "b2r": shared["b2r"], "dslot": pc["dslot"],
                 "sarr": pc["sarr"], "idx1": pc["idx1"], "idx2": pc["idx2"]}
            m.update(extra(c))
            ms.append(m)
        return ms

    resA = run_bass_kernel_spmd(ncA, maps(lambda c: {}), core_ids=list(range(NCORES)))
    t2f = np.concatenate([resA.results[c]["t2l"] for c in range(NCORES)], 0)
    resB = run_bass_kernel_spmd(ncB, maps(lambda c: {"t2f": t2f}),
                                core_ids=list(range(NCORES)))
    full = np.empty((N_NODES, OUT_CH), np.float32)
    for c in range(NCORES):
        full[c * SHARD + per_core[c]["order"]] = resB.results[c]["out"][:SHARD]
    return full
